# revision 12
# baseline (speedup 1.0000x reference)
"""Trainium2 Bass kernel for nn_Custom_trainer_79242146611896.

Data-parallel over N=16384 samples across 8 NeuronCores (2048/core).
v2 design (vs baseline):
  - bf16 matmul operands everywhere (tolerance 2e-2 >> bf16 error).
  - mm2 (decoded) emits NATURAL layout via stat=encT slices, so the rec
    pinball subtracts `output` rows directly -- no dct transposes.
  - mm4 uses the precomputed W_dd = W_dec @ W_enc ([512,512], built
    on-device), turning the [N,T]@[T,D] matmul into [N,D]@[D,D]:
    131k -> 33k PE rows; output in latT layout so the b_enc bias is a
    free per-partition activation bias and the lat diff is elementwise
    against encT.
  - two collectives: seg-sums+counts all-reduce launched right after
    phase A (hidden behind phase B compute); tiny scalar-partials
    all-reduce launched after phase B (hidden behind phase C).
  - pinball |diff| accumulated per [128,512] tile: vector subtract
    (in-place on PSUM) + scalar Abs activation with accum_out.
  - CCE computed blocked 4 samples-tiles wide ([128,200]) with the
    log-domain clip:  log(clip(yp,eps,1)) = max(log(cl)-log(rs), log eps).
  - epilogue fused: eps matmul carries a -0.5*msq K=1 row so
    wgss = (nsq + sum_c oh*(-2*eps'))/D + S with two vector ops per
    sample-tile and ONE final [128,16] op + one output DMA.
"""

import numpy as np

import concourse.bass as bass
import concourse.mybir as mybir
import concourse.tile as tile
from concourse import bacc
from concourse.bass_utils import run_bass_kernel_spmd
from concourse.masks import make_identity

F32 = mybir.dt.float32
F32R = mybir.dt.float32r
BF16 = mybir.dt.bfloat16
I32 = mybir.dt.int32
AX = mybir.AxisListType
ALU = mybir.AluOpType
ACTF = mybir.ActivationFunctionType

P = 128
NCORES = 8
N_GLOBAL = 16384
T = 2048
D = 512
C = 50
KEPS = 1e-7
LNEPS = float(np.log(KEPS))


def build(nl=N_GLOBAL // NCORES, nc_chunk=512, n_global=None):
    n_global = n_global or NCORES * nl
    NT = T // P          # 16 T-tiles
    ND = D // P          # 4 D-tiles
    NN = nl // P         # 16 sample-tiles per core
    NC = nc_chunk        # samples per chunk (512)
    NCH = nl // NC       # 4 chunks
    NSUB = NC // P       # 4 sample-tiles per chunk
    NQ = T // 512        # 4 columns-of-512 per T row

    nc = bacc.Bacc("TRN2", target_bir_lowering=False, debug=False, num_devices=NCORES)

    x_d = nc.dram_tensor("x", [nl, T], F32, kind="ExternalInput")
    o_d = nc.dram_tensor("output", [nl, T], F32, kind="ExternalInput")
    cl_d = nc.dram_tensor("cat_labels", [nl, C], F32, kind="ExternalInput")
    lab_d = nc.dram_tensor("labels", [nl], I32, kind="ExternalInput")
    wenc_d = nc.dram_tensor("W_enc", [T, D], F32, kind="ExternalInput")
    benc_d = nc.dram_tensor("b_enc", [D], F32, kind="ExternalInput")
    wdec_d = nc.dram_tensor("W_dec", [D, T], F32, kind="ExternalInput")
    bdec_d = nc.dram_tensor("b_dec", [T], F32, kind="ExternalInput")
    wcls_d = nc.dram_tensor("W_cls", [D, C], F32, kind="ExternalInput")
    bcls_d = nc.dram_tensor("b_cls", [C], F32, kind="ExternalInput")
    out_d = nc.dram_tensor("out", [nl], F32, kind="ExternalOutput")

    from contextlib import ExitStack

    with tile.TileContext(nc) as tc:
        with ExitStack() as ctx:
            ent = ctx.enter_context
            constp = ent(tc.tile_pool(name="const", bufs=1))
            wts = ent(tc.tile_pool(name="wts", bufs=1))
            encp = ent(tc.tile_pool(name="enc", bufs=1))
            accp = ent(tc.tile_pool(name="acc", bufs=1))
            rows = ent(tc.tile_pool(name="rows", bufs=7))
            xtp = ent(tc.tile_pool(name="xt", bufs=16))
            enp = ent(tc.tile_pool(name="enp", bufs=4))
            wdtp = ent(tc.tile_pool(name="wdt", bufs=1))
            junkp = ent(tc.tile_pool(name="junk", bufs=2))
            ccep = ent(tc.tile_pool(name="cce", bufs=4))
            smallp = ent(tc.tile_pool(name="small", bufs=10))
            colsp = ent(tc.tile_pool(name="cols", bufs=2))
            psA = ent(tc.tile_pool(name="psA", bufs=2, space="PSUM"))
            psT = ent(tc.tile_pool(name="psT", bufs=2, space="PSUM"))
            psS = ent(tc.tile_pool(name="psS", bufs=2, space="PSUM"))
            psSeg = ent(tc.tile_pool(name="psSeg", bufs=1, space="PSUM"))
            psCnt = ent(tc.tile_pool(name="psCnt", bufs=1, space="PSUM"))
            dp = ent(tc.tile_pool(name="dram", bufs=1, space="DRAM"))

            # ---------------- constants ----------------
            ident_f32 = constp.tile([P, P], F32)
            make_identity(nc, ident_f32)
            ident_bf = constp.tile([P, P], BF16)
            nc.vector.tensor_copy(ident_bf[:], ident_f32[:])
            ones_col_f = constp.tile([P, 1], F32)
            nc.any.memset(ones_col_f[:], 1.0)
            ones_col_bf = constp.tile([P, 1], BF16)
            nc.vector.tensor_copy(ones_col_bf[:], ones_col_f[:])
            ones_k1f = constp.tile([1, P], F32)
            nc.any.memset(ones_k1f[:], 1.0)
            ones_k1bf = constp.tile([1, P], BF16)
            nc.vector.tensor_copy(ones_k1bf[:], ones_k1f[:])

            iot = constp.tile([P, C], I32)
            nc.gpsimd.iota(iot[:], [[1, C]], channel_multiplier=0)
            iotaf = constp.tile([P, C], F32)
            nc.vector.tensor_copy(iotaf[:], iot[:])

            # ---------------- chunk-0 x loads first ----------------
            xr_tiles = {}

            def load_chunk_x(c):
                for s in range(NSUB):
                    r_ = rows.tile([P, T], F32, name="xrow", tag="row")
                    nc.sync.dma_start(
                        r_[:], x_d[c * NC + s * P : c * NC + (s + 1) * P, :]
                    )
                    xr_tiles[(c, s)] = r_

            load_chunk_x(0)
            _defer_c1 = True

            # ---------------- weights (load f32, cast bf16) ----------------
            def load_cast(shape, dram_ap, tag):
                s_ = rows.tile([P, T], F32, name="stg", tag="row")[: shape[0], : shape[1]]
                nc.sync.dma_start(s_[:], dram_ap)
                t_ = wts.tile(shape, BF16, name=tag, tag=tag)
                nc.scalar.activation(t_[:], s_[:], ACTF.Copy)
                return t_

            wenc_r = wenc_d.ap().rearrange("(a p) d -> a p d", p=P)
            wenc = [load_cast([P, D], wenc_r[t], f"wenc{t}") for t in range(NT)]
            load_chunk_x(1)
            wdec_r = wdec_d.ap().rearrange("(a p) t -> a p t", p=P)
            wdec = [load_cast([P, T], wdec_r[k], f"wdec{k}") for k in range(ND)]
            wcls_r = wcls_d.ap().rearrange("(a p) c -> a p c", p=P)
            wcls = [load_cast([P, C], wcls_r[k], f"wcls{k}") for k in range(ND)]
            bcls_row = load_cast([1, C], bcls_d.ap().rearrange("(o c) -> o c", o=1), "bcls_row")

            benc_r = benc_d.ap().rearrange("(a p) -> a p", p=P)
            bencT = []
            for k in range(ND):
                b_ = wts.tile([P, 1], F32, name="bencT", tag=f"bencT{k}")
                nc.sync.dma_start(b_[:], benc_r[k].rearrange("(p o) -> p o", o=1))
                bencT.append(b_)

            # bdec broadcast to all partitions via K=1 f32 matmuls
            bdec_row_t = rows.tile([P, T], F32, name="bdec_row", tag="row")
            bdec_row = bdec_row_t[0:1, :]
            nc.sync.dma_start(bdec_row, bdec_d.ap().rearrange("(o t) -> o t", o=1))
            bdec16 = wts.tile([1, T], BF16, name="bdec16", tag="bdec16")
            nc.vector.tensor_copy(bdec16[:], bdec_row)

            # ---------------- persistent state ----------------
            encT = [encp.tile([P, nl], BF16, name=f"encT{k}", tag=f"encT{k}") for k in range(ND)]
            rec_strip = accp.tile([P, NN * NQ], F32)
            lat_strip = accp.tile([P, NCH * ND], F32)
            cat_strip = accp.tile([P, NN], F32)
            nsq_strip = accp.tile([P, NN], F32)
            gq_strip = accp.tile([P, NN], F32)
            onehot_f = [accp.tile([P, C], F32, name=f"ohf{i}", tag=f"ohf{i}") for i in range(NN)]
            onehot_b = [accp.tile([P, C], BF16, name=f"ohb{i}", tag=f"ohb{i}") for i in range(NN)]

            seg_ps = psSeg.tile([C, D], F32, name="seg_ps", tag="seg")
            cnt_ps = psCnt.tile([C, 1], F32, name="cnt_ps", tag="cnt")

            # labels: one contiguous row load, PE-transpose to per-partition cols
            labrow_f = rows.tile([P, T], F32, name="labrow", tag="row")
            labrow = labrow_f[0:1, 0:nl].bitcast(I32)
            nc.sync.dma_start(labrow, lab_d.ap().rearrange("(o n) -> o n", o=1))
            lab_ps = psS.tile([P, NN], F32, name="lab_ps", tag="psS")
            for i in range(NN):
                nc.tensor.transpose(
                    lab_ps[:, i : i + 1],
                    labrow[:, i * P : (i + 1) * P].bitcast(F32),
                    ident_f32[0:1, 0:1],
                )
            labf_strip = accp.tile([P, NN], F32)
            nc.vector.tensor_copy(labf_strip[:], lab_ps[:].bitcast(I32))

            # ================= phase A =================
            or_tiles = {}

            def load_chunk_o(c):
                for s in range(NSUB):
                    r_ = rows.tile([P, T], F32, name="orow", tag="row")
                    nc.sync.dma_start(r_[:], o_d[c * NC + s * P : c * NC + (s + 1) * P, :])
                    or_tiles[(c, s)] = r_

            for c in range(NCH):
                if c + 2 < NCH:
                    load_chunk_x(c + 2)
                elif c + 2 == NCH:
                    load_chunk_o(0)
                base = c * NC
                xr = [xr_tiles.pop((c, s)) for s in range(NSUB)]

                # transpose x (f32, 2c/r); evacuation casts to bf16
                xt = []
                for t in range(NT):
                    tp = psT.tile([P, NC], F32, name="tp", tag="psT")
                    for s in range(NSUB):
                        nc.tensor.transpose(
                            tp[:, s * P : (s + 1) * P],
                            xr[s][:, t * P : (t + 1) * P],
                            ident_f32,
                        )
                    xt_t = xtp.tile([P, NC], BF16, name="xt", tag="xt")
                    if t % 2 == 0:
                        nc.scalar.activation(xt_t[:], tp[:], ACTF.Copy)
                    else:
                        nc.vector.tensor_copy(xt_t[:], tp[:])
                    xt.append(xt_t)

                # mm1: encodedT[:, chunk] = tanh(W_enc^T xT + b_enc)
                for k in range(ND):
                    ps = psA.tile([P, NC], F32, name="ps1", tag="psA")
                    for t in range(NT):
                        nc.tensor.matmul(
                            ps[:], wenc[t][:, k * P : (k + 1) * P], xt[t][:],
                            start=(t == 0), stop=(t == NT - 1),
                        )
                    nc.scalar.activation(
                        encT[k][:, base : base + NC], ps[:], ACTF.Tanh, bias=bencT[k][:]
                    )

                # enc natural tiles + onehot + seg/cnt accumulation + nsq
                for s in range(NSUB):
                    i = c * NSUB + s
                    pse = psT.tile([P, D], BF16, name="pseb", tag="psT")
                    for k in range(ND):
                        nc.tensor.transpose(
                            pse[:, k * P : (k + 1) * P],
                            encT[k][:, base + s * P : base + (s + 1) * P],
                            ident_bf,
                        )
                    en = enp.tile([P, D], BF16, name="en", tag="en")
                    nc.vector.tensor_copy(en[:], pse[:])

                    nc.vector.tensor_scalar(
                        out=onehot_f[i][:], in0=iotaf[:], scalar1=labf_strip[:, i : i + 1],
                        scalar2=None, op0=ALU.is_equal,
                    )
                    nc.vector.tensor_copy(onehot_b[i][:], onehot_f[i][:])

                    nc.tensor.matmul(
                        seg_ps[:], onehot_b[i][:], en[:],
                        start=(i == 0), stop=(i == NN - 1),
                    )
                    nc.tensor.matmul(
                        cnt_ps[:], onehot_b[i][:], ones_col_bf[:],
                        start=(i == 0), stop=(i == NN - 1),
                    )

                    jn = junkp.tile([P, D], F32, name="jn", tag="junk")
                    nc.vector.scalar_tensor_tensor(
                        out=jn[:], in0=en[:], scalar=0.0, in1=en[:],
                        op0=ALU.bypass, op1=ALU.mult,
                        accum_out=nsq_strip[:, i : i + 1],
                    )

            # ============ collective #1: seg sums + counts ============
            seg_sb = accp.tile([C, D], F32)
            nc.vector.tensor_copy(seg_sb[:], seg_ps[:])
            cnt_sb = accp.tile([C, 1], F32)
            nc.vector.tensor_copy(cnt_sb[:], cnt_ps[:])

            bounce1_in = dp.tile([C, D + 1], F32, name="bounce1_in", tag="b1i")
            bounce1_out = dp.tile([C, D + 1], F32, name="bounce1_out", tag="b1o")
            nc.sync.dma_start(bounce1_in[:, 0:D], seg_sb[:])
            nc.sync.dma_start(bounce1_in[:, D : D + 1], cnt_sb[:])
            nc.gpsimd.collective_compute(
                "AllReduce",
                ALU.add,
                replica_groups=[list(range(NCORES))],
                ins=[bounce1_in[:].opt()],
                outs=[bounce1_out[:].opt()],
            )
            # ============ W_dd = W_dec @ W_enc (on device, bf16) ============
            wdecT = []
            for t in range(NT):
                psw = psT.tile([P, D], BF16, name="psw", tag="psT")
                for k in range(ND):
                    nc.tensor.transpose(
                        psw[:, k * P : (k + 1) * P],
                        wdec[k][:, t * P : (t + 1) * P],
                        ident_bf,
                    )
                wt_ = wdtp.tile([P, D], BF16, name="wdecT", tag=f"wdecT{t}")
                nc.vector.tensor_copy(wt_[:], psw[:])
                wdecT.append(wt_)
            W_dd = []
            for k in range(ND):
                psd = psA.tile([P, D], F32, name="psd", tag="psA")
                for t in range(NT):
                    nc.tensor.matmul(
                        psd[:], wdecT[t][:, k * P : (k + 1) * P], wenc[t][:],
                        start=(t == 0), stop=(t == NT - 1),
                    )
                wd_ = wts.tile([P, D], BF16, name="W_dd", tag=f"Wdd{k}")
                nc.scalar.activation(wd_[:], psd[:], ACTF.Copy)
                W_dd.append(wd_)

            # ================= phase B =================
            for c in range(NCH):
                base = c * NC

                if c + 1 < NCH:
                    load_chunk_o(c + 1)
                orow = [or_tiles.pop((c, s)) for s in range(NSUB)]

                # mm2 natural + rec pinball
                for s in range(NSUB):
                    i = c * NSUB + s
                    for q in range(NQ):
                        pool = psA if q % 2 == 0 else psT
                        ps = pool.tile([P, 512], F32, name="ps2", tag=pool.name)
                        nc.tensor.matmul(
                            ps[:], ones_k1bf[:], bdec16[:, q * 512 : (q + 1) * 512],
                            start=True, stop=False,
                        )
                        for k in range(ND):
                            nc.tensor.matmul(
                                ps[:],
                                encT[k][:, i * P : (i + 1) * P],
                                wdec[k][:, q * 512 : (q + 1) * 512],
                                start=False, stop=(k == ND - 1),
                            )
                        nc.vector.tensor_tensor(
                            ps[:], ps[:], orow[s][:, q * 512 : (q + 1) * 512],
                            ALU.subtract,
                        )
                        nc.scalar.activation(
                            ps[:], ps[:], ACTF.Abs,
                            accum_out=rec_strip[:, i * NQ + q : i * NQ + q + 1],
                        )

                # mm4: rec_latentsT via W_dd; lat pinball vs encT
                for k2 in range(ND):
                    ps = psA.tile([P, NC], F32, name="ps4", tag="psA")
                    for k in range(ND):
                        nc.tensor.matmul(
                            ps[:],
                            W_dd[k][:, k2 * P : (k2 + 1) * P],
                            encT[k][:, base : base + NC],
                            start=(k == 0), stop=(k == ND - 1),
                        )
                    nc.scalar.activation(ps[:], ps[:], ACTF.Tanh, bias=bencT[k2][:])
                    nc.vector.tensor_tensor(
                        ps[:], ps[:], encT[k2][:, base : base + NC], ALU.subtract
                    )
                    nc.vector.tensor_reduce(
                        lat_strip[:, c * ND + k2 : c * ND + k2 + 1], ps[:],
                        AX.X, ALU.add, apply_absolute_value=True,
                    )

                # mm3 + CCE, blocked [128, 4*50]
                psl = psS.tile([P, NSUB * C], F32, name="psl", tag="psS")
                for a in range(NSUB):
                    sl = slice(a * C, (a + 1) * C)
                    nc.tensor.matmul(
                        psl[:, sl], ones_k1bf[:], bcls_row[:],
                        start=True, stop=False,
                    )
                    for k in range(ND):
                        nc.tensor.matmul(
                            psl[:, sl],
                            encT[k][:, (base + a * P) : (base + (a + 1) * P)],
                            wcls[k][:],
                            start=False, stop=(k == ND - 1),
                        )
                expt = ccep.tile([P, NSUB * C], F32, name="expt", tag="cce")
                nc.scalar.activation(expt[:], psl[:], ACTF.Exp)
                sume = smallp.tile([P, NSUB], F32, name="sume", tag="small")
                nc.vector.tensor_reduce(
                    sume[:], expt[:].rearrange("p (a c) -> p a c", c=C),
                    AX.X, ALU.add,
                )
                rcp = smallp.tile([P, NSUB], F32, name="rcp", tag="small")
                nc.vector.reciprocal(rcp[:], sume[:])

                cl = ccep.tile([P, NSUB * C], F32, name="cl", tag="cce")
                for a in range(NSUB):
                    nc.sync.dma_start(
                        cl[:, a * C : (a + 1) * C],
                        cl_d[(base + a * P) : (base + (a + 1) * P), :],
                    )
                rs = smallp.tile([P, NSUB], F32, name="rs", tag="small")
                nc.vector.tensor_reduce(
                    rs[:], cl[:].rearrange("p (a c) -> p a c", c=C),
                    AX.X, ALU.add,
                )
                lnrs = smallp.tile([P, NSUB], F32, name="lnrs", tag="small")
                nc.scalar.activation(lnrs[:], rs[:], ACTF.Ln)
                lg = ccep.tile([P, NSUB * C], F32, name="lg", tag="cce")
                nc.scalar.activation(lg[:], cl[:], ACTF.Ln)
                for a in range(NSUB):
                    i = c * NSUB + a
                    sl = slice(a * C, (a + 1) * C)
                    lgc = ccep.tile([P, C], F32, name="lgc", tag="lgc")
                    nc.vector.tensor_scalar(
                        out=lgc[:], in0=lg[:, sl], scalar1=lnrs[:, a : a + 1],
                        scalar2=LNEPS, op0=ALU.subtract, op1=ALU.max,
                    )
                    jc = ccep.tile([P, C], F32, name="jc", tag="lgc")
                    nc.vector.scalar_tensor_tensor(
                        out=jc[:], in0=expt[:, sl], scalar=rcp[:, a : a + 1],
                        in1=lgc[:], op0=ALU.mult, op1=ALU.mult,
                        accum_out=cat_strip[:, i : i + 1],
                    )

            # readback of collective #1 (emitted late so the in-order DMA
            # queue never blocks phase-B streaming loads on the collective)
            sums_g = accp.tile([C, D], F32)
            nc.sync.dma_start(sums_g[:], bounce1_out[:, 0:D])
            counts_g = accp.tile([C, 1], F32)
            nc.sync.dma_start(counts_g[:], bounce1_out[:, D : D + 1])

            # ============ collective #2: scalar partials ============
            pack3 = accp.tile([P, 3], F32)
            nc.vector.tensor_reduce(pack3[:, 0:1], rec_strip[:], AX.X, ALU.add)
            nc.vector.tensor_reduce(pack3[:, 1:2], lat_strip[:], AX.X, ALU.add)
            nc.vector.tensor_reduce(pack3[:, 2:3], cat_strip[:], AX.X, ALU.add)
            scps = psS.tile([1, 3], F32, name="scps", tag="psS")
            nc.tensor.matmul(scps[:], ones_col_f[:], pack3[:], start=True, stop=True)
            sc_row = accp.tile([1, 128], F32)
            nc.any.memset(sc_row[:], 0.0)
            nc.vector.tensor_copy(sc_row[:, 0:3], scps[:])

            bounce2_in = dp.tile([1, 128], F32, name="bounce2_in", tag="b2i")
            bounce2_out = dp.tile([1, 128], F32, name="bounce2_out", tag="b2o")
            nc.sync.dma_start(bounce2_in[:], sc_row[:])
            nc.gpsimd.collective_compute(
                "AllReduce",
                ALU.add,
                replica_groups=[list(range(NCORES))],
                ins=[bounce2_in[:].opt()],
                outs=[bounce2_out[:].opt()],
            )
            sc_g = accp.tile([1, 3], F32)
            nc.sync.dma_start(sc_g[:], bounce2_out[:, 0:3])

            # ================= phase C =================
            cmax = accp.tile([C, 1], F32)
            nc.vector.tensor_scalar(
                out=cmax[:], in0=counts_g[:], scalar1=1.0, scalar2=None, op0=ALU.max
            )
            crcp = accp.tile([C, 1], F32)
            nc.vector.reciprocal(crcp[:], cmax[:])
            means = accp.tile([C, D], F32)
            nc.vector.tensor_scalar(
                out=means[:], in0=sums_g[:], scalar1=crcp[:], scalar2=None, op0=ALU.mult
            )
            msq_col = accp.tile([C, 1], F32)
            jm = junkp.tile([C, D], F32, name="jm", tag="junk")
            nc.vector.scalar_tensor_tensor(
                out=jm[:], in0=means[:], scalar=0.0, in1=means[:],
                op0=ALU.bypass, op1=ALU.mult, accum_out=msq_col[:],
            )

            meansT = []
            for k in range(ND):
                tpm = psS.tile([P, C], F32, name="tpm", tag="psS")
                nc.tensor.transpose(
                    tpm[:], means[:, k * P : (k + 1) * P], ident_f32[:C, :C]
                )
                mt = accp.tile([P, C], BF16, name="meansT", tag=f"meansT{k}")
                nc.vector.tensor_copy(mt[:], tpm[:])
                meansT.append(mt)

            tpq = psS.tile([1, C], F32, name="tpq", tag="psS")
            nc.tensor.transpose(tpq[:], msq_col[:], ident_f32[:C, :C])
            msqm05_row = accp.tile([1, C], BF16)
            nc.scalar.activation(msqm05_row[:], tpq[:], ACTF.Copy, scale=-0.5)

            # eps' = enc @ means^T - 0.5*msq ; gq = -2 * sum_c oh * eps'
            for g in range(NN // NSUB):
                pse2 = psS.tile([P, NSUB * C], F32, name="pse2", tag="psS")
                for a in range(NSUB):
                    i = g * NSUB + a
                    sl = slice(a * C, (a + 1) * C)
                    nc.tensor.matmul(
                        pse2[:, sl], ones_k1bf[:], msqm05_row[:],
                        start=True, stop=False,
                    )
                    for k in range(ND):
                        nc.tensor.matmul(
                            pse2[:, sl],
                            encT[k][:, i * P : (i + 1) * P],
                            meansT[k][:],
                            start=False, stop=(k == ND - 1),
                        )
                for a in range(NSUB):
                    i = g * NSUB + a
                    sl = slice(a * C, (a + 1) * C)
                    jq = ccep.tile([P, C], F32, name="jq", tag="lgc")
                    nc.vector.scalar_tensor_tensor(
                        out=jq[:], in0=pse2[:, sl], scalar=-2.0, in1=onehot_f[i][:],
                        op0=ALU.mult, op1=ALU.mult,
                        accum_out=gq_strip[:, i : i + 1],
                    )

            # ---- final: S scalar + combine + output DMA ----
            coef = accp.tile([1, 3], F32)
            nc.any.memset(coef[:, 0:1], 0.9 / (n_global * T))
            nc.any.memset(coef[:, 1:2], 0.9 / (n_global * D))
            nc.any.memset(coef[:, 2:3], -1.0 / n_global)
            sprod = accp.tile([1, 3], F32)
            nc.vector.tensor_tensor(sprod[:], sc_g[:], coef[:], ALU.mult)
            stot = accp.tile([1, 1], F32)
            nc.vector.tensor_reduce(stot[:], sprod[:], AX.X, ALU.add)
            psS1 = psS.tile([P, 1], F32, name="psS1", tag="psS")
            nc.tensor.matmul(psS1[:], ones_k1f[:], stot[:], start=True, stop=True)
            s_col = accp.tile([P, 1], F32)
            nc.vector.tensor_copy(s_col[:], psS1[:])

            ns2 = accp.tile([P, NN], F32)
            nc.vector.tensor_scalar(
                out=ns2[:], in0=nsq_strip[:], scalar1=1.0 / D, scalar2=s_col[:],
                op0=ALU.mult, op1=ALU.add,
            )
            out_strip = accp.tile([P, NN], F32)
            nc.vector.scalar_tensor_tensor(
                out=out_strip[:], in0=gq_strip[:], scalar=1.0 / D, in1=ns2[:],
                op0=ALU.mult, op1=ALU.add,
            )
            ps_out = psS.tile([NN, P], F32, name="ps_out", tag="psS")
            nc.tensor.transpose(ps_out[:], out_strip[:], ident_f32)
            outT = accp.tile([NN, P], F32)
            nc.vector.tensor_copy(outT[:], ps_out[:])
            nc.sync.dma_start(
                out_d.ap().rearrange("(a p) -> a p", p=P), outT[:]
            )

    nc.compile()
    return nc


_CACHE = {}


def _get_nc():
    if "nc" not in _CACHE:
        _CACHE["nc"] = build()
    return _CACHE["nc"]


def kernel(**inputs):
    nc = _get_nc()
    nl = N_GLOBAL // NCORES
    shard_names = ["x", "output", "cat_labels", "labels"]
    full_names = ["W_enc", "b_enc", "W_dec", "b_dec", "W_cls", "b_cls"]
    in_maps = []
    for i in range(NCORES):
        m = {}
        for k in shard_names:
            m[k] = np.ascontiguousarray(inputs[k][i * nl : (i + 1) * nl])
        for k in full_names:
            m[k] = np.ascontiguousarray(inputs[k])
        in_maps.append(m)
    res = run_bass_kernel_spmd(nc, in_maps, list(range(NCORES))).results
    return np.concatenate([res[i]["out"] for i in range(NCORES)]).astype(np.float32)


# revision 14
# speedup vs baseline: 1.1199x; 1.1199x over previous
"""Trainium2 Bass kernel for nn_Custom_trainer_79242146611896.

Data-parallel over N=16384 samples across 8 NeuronCores (2048/core).
v2 design (vs baseline):
  - bf16 matmul operands everywhere (tolerance 2e-2 >> bf16 error).
  - mm2 (decoded) emits NATURAL layout via stat=encT slices, so the rec
    pinball subtracts `output` rows directly -- no dct transposes.
  - mm4 uses the precomputed W_dd = W_dec @ W_enc ([512,512], built
    on-device), turning the [N,T]@[T,D] matmul into [N,D]@[D,D]:
    131k -> 33k PE rows; output in latT layout so the b_enc bias is a
    free per-partition activation bias and the lat diff is elementwise
    against encT.
  - two collectives: seg-sums+counts all-reduce launched right after
    phase A (hidden behind phase B compute); tiny scalar-partials
    all-reduce launched after phase B (hidden behind phase C).
  - pinball |diff| accumulated per [128,512] tile: vector subtract
    (in-place on PSUM) + scalar Abs activation with accum_out.
  - CCE computed blocked 4 samples-tiles wide ([128,200]) with the
    log-domain clip:  log(clip(yp,eps,1)) = max(log(cl)-log(rs), log eps).
  - epilogue fused: eps matmul carries a -0.5*msq K=1 row so
    wgss = (nsq + sum_c oh*(-2*eps'))/D + S with two vector ops per
    sample-tile and ONE final [128,16] op + one output DMA.
"""

import numpy as np

import concourse.bass as bass
import concourse.mybir as mybir
import concourse.tile as tile
from concourse import bacc
from concourse.bass_utils import run_bass_kernel_spmd
from concourse.masks import make_identity

F32 = mybir.dt.float32
F32R = mybir.dt.float32r
BF16 = mybir.dt.bfloat16
I32 = mybir.dt.int32
AX = mybir.AxisListType
ALU = mybir.AluOpType
ACTF = mybir.ActivationFunctionType

P = 128
NCORES = 8
N_GLOBAL = 16384
T = 2048
D = 512
C = 50
KEPS = 1e-7
LNEPS = float(np.log(KEPS))


def build(nl=N_GLOBAL // NCORES, nc_chunk=512, n_global=None, with_bdec=True):
    n_global = n_global or NCORES * nl
    NT = T // P          # 16 T-tiles
    ND = D // P          # 4 D-tiles
    NN = nl // P         # 16 sample-tiles per core
    NC = nc_chunk        # samples per chunk (512)
    NCH = nl // NC       # 4 chunks
    NSUB = NC // P       # 4 sample-tiles per chunk
    NQ = T // 512        # 4 columns-of-512 per T row

    nc = bacc.Bacc("TRN2", target_bir_lowering=False, debug=False, num_devices=NCORES)

    x_d = nc.dram_tensor("x", [nl, T], F32, kind="ExternalInput")
    o_d = nc.dram_tensor("output", [nl, T], F32, kind="ExternalInput")
    cl_d = nc.dram_tensor("cat_labels", [nl, C], F32, kind="ExternalInput")
    lab_d = nc.dram_tensor("labels", [nl], I32, kind="ExternalInput")
    wenc_d = nc.dram_tensor("W_enc", [T, D], F32, kind="ExternalInput")
    benc_d = nc.dram_tensor("b_enc", [D], F32, kind="ExternalInput")
    wdec_d = nc.dram_tensor("W_dec", [D, T], F32, kind="ExternalInput")
    bdec_d = nc.dram_tensor("b_dec", [T], F32, kind="ExternalInput")
    wcls_d = nc.dram_tensor("W_cls", [D, C], F32, kind="ExternalInput")
    bcls_d = nc.dram_tensor("b_cls", [C], F32, kind="ExternalInput")
    out_d = nc.dram_tensor("out", [nl], F32, kind="ExternalOutput")

    from contextlib import ExitStack

    with tile.TileContext(nc) as tc:
        with ExitStack() as ctx:
            ent = ctx.enter_context
            constp = ent(tc.tile_pool(name="const", bufs=1))
            wts = ent(tc.tile_pool(name="wts", bufs=1))
            encp = ent(tc.tile_pool(name="enc", bufs=1))
            accp = ent(tc.tile_pool(name="acc", bufs=1))
            rows = ent(tc.tile_pool(name="rows", bufs=7))
            xtp = ent(tc.tile_pool(name="xt", bufs=16))
            enp = ent(tc.tile_pool(name="enp", bufs=4))
            wdtp = ent(tc.tile_pool(name="wdt", bufs=1))
            junkp = ent(tc.tile_pool(name="junk", bufs=2))
            ccep = ent(tc.tile_pool(name="cce", bufs=4))
            smallp = ent(tc.tile_pool(name="small", bufs=10))
            colsp = ent(tc.tile_pool(name="cols", bufs=2))
            psA = ent(tc.tile_pool(name="psA", bufs=3, space="PSUM"))
            psT = ent(tc.tile_pool(name="psT", bufs=2, space="PSUM"))
            psS = ent(tc.tile_pool(name="psS", bufs=1, space="PSUM"))
            psSeg = ent(tc.tile_pool(name="psSeg", bufs=1, space="PSUM"))
            psCnt = ent(tc.tile_pool(name="psCnt", bufs=1, space="PSUM"))
            dp = ent(tc.tile_pool(name="dram", bufs=1, space="DRAM"))

            # ---------------- constants ----------------
            ident_f32 = constp.tile([P, P], F32)
            make_identity(nc, ident_f32)
            ident_bf = constp.tile([P, P], BF16)
            nc.vector.tensor_copy(ident_bf[:], ident_f32[:])
            ones_col_f = constp.tile([P, 1], F32)
            nc.any.memset(ones_col_f[:], 1.0)
            ones_col_bf = constp.tile([P, 1], BF16)
            nc.vector.tensor_copy(ones_col_bf[:], ones_col_f[:])
            ones_k1f = constp.tile([1, P], F32)
            nc.any.memset(ones_k1f[:], 1.0)
            ones_k1bf = constp.tile([1, P], BF16)
            nc.vector.tensor_copy(ones_k1bf[:], ones_k1f[:])

            iot = constp.tile([P, C], I32)
            nc.gpsimd.iota(iot[:], [[1, C]], channel_multiplier=0)
            iotaf = constp.tile([P, C], F32)
            nc.vector.tensor_copy(iotaf[:], iot[:])

            # ---------------- chunk-0 x loads first ----------------
            xr_tiles = {}

            def load_chunk_x(c):
                for s in range(NSUB):
                    r_ = rows.tile([P, T], F32, name="xrow", tag="row")
                    nc.sync.dma_start(
                        r_[:], x_d[c * NC + s * P : c * NC + (s + 1) * P, :]
                    )
                    xr_tiles[(c, s)] = r_

            load_chunk_x(0)
            _defer_c1 = True

            # ---------------- weights (load f32, cast bf16) ----------------
            def load_cast(shape, dram_ap, tag):
                s_ = rows.tile([P, T], F32, name="stg", tag="row")[: shape[0], : shape[1]]
                nc.sync.dma_start(s_[:], dram_ap)
                t_ = wts.tile(shape, BF16, name=tag, tag=tag)
                nc.scalar.activation(t_[:], s_[:], ACTF.Copy)
                return t_

            wenc_r = wenc_d.ap().rearrange("(a p) d -> a p d", p=P)
            wenc = [load_cast([P, D], wenc_r[t], f"wenc{t}") for t in range(NT)]
            load_chunk_x(1)
            wdec_r = wdec_d.ap().rearrange("(a p) t -> a p t", p=P)
            wdec = [load_cast([P, T], wdec_r[k], f"wdec{k}") for k in range(ND)]
            wcls_r = wcls_d.ap().rearrange("(a p) c -> a p c", p=P)
            wcls = [load_cast([P, C], wcls_r[k], f"wcls{k}") for k in range(ND)]
            bcls_row = load_cast([1, C], bcls_d.ap().rearrange("(o c) -> o c", o=1), "bcls_row")

            benc_r = benc_d.ap().rearrange("(a p) -> a p", p=P)
            bencT = []
            for k in range(ND):
                b_ = wts.tile([P, 1], F32, name="bencT", tag=f"bencT{k}")
                nc.sync.dma_start(b_[:], benc_r[k].rearrange("(p o) -> p o", o=1))
                bencT.append(b_)

            # bdec broadcast to all partitions via K=1 f32 matmuls
            bdec_row_t = rows.tile([P, T], F32, name="bdec_row", tag="row")
            bdec_row = bdec_row_t[0:1, :]
            nc.sync.dma_start(bdec_row, bdec_d.ap().rearrange("(o t) -> o t", o=1))
            bdec16 = wts.tile([1, T], BF16, name="bdec16", tag="bdec16")
            nc.vector.tensor_copy(bdec16[:], bdec_row)

            # ---------------- persistent state ----------------
            encT = [encp.tile([P, nl], BF16, name=f"encT{k}", tag=f"encT{k}") for k in range(ND)]
            rec_strip = accp.tile([P, NN * NQ], F32)
            lat_strip = accp.tile([P, NCH * ND], F32)
            cat_strip = accp.tile([P, NN], F32)
            nsq_strip = accp.tile([P, NN], F32)
            gq_strip = accp.tile([P, NN], F32)
            onehot_f = [accp.tile([P, C], F32, name=f"ohf{i}", tag=f"ohf{i}") for i in range(NN)]
            onehot_b = [accp.tile([P, C], BF16, name=f"ohb{i}", tag=f"ohb{i}") for i in range(NN)]

            seg_ps = psSeg.tile([C, D], F32, name="seg_ps", tag="seg")
            cnt_ps = psCnt.tile([C, 1], F32, name="cnt_ps", tag="cnt")

            # labels: one contiguous row load, PE-transpose to per-partition cols
            labrow_f = rows.tile([P, T], F32, name="labrow", tag="row")
            labrow = labrow_f[0:1, 0:nl].bitcast(I32)
            nc.sync.dma_start(labrow, lab_d.ap().rearrange("(o n) -> o n", o=1))
            lab_ps = psS.tile([P, NN], F32, name="lab_ps", tag="psS")
            for i in range(NN):
                nc.tensor.transpose(
                    lab_ps[:, i : i + 1],
                    labrow[:, i * P : (i + 1) * P].bitcast(F32),
                    ident_f32[0:1, 0:1],
                )
            labf_strip = accp.tile([P, NN], F32)
            nc.vector.tensor_copy(labf_strip[:], lab_ps[:].bitcast(I32))

            # ================= phase A =================
            or_tiles = {}

            def load_chunk_o(c):
                for s in range(NSUB):
                    r_ = rows.tile([P, T], F32, name="orow", tag="row")
                    nc.sync.dma_start(r_[:], o_d[c * NC + s * P : c * NC + (s + 1) * P, :])
                    or_tiles[(c, s)] = r_

            for c in range(NCH):
                if c + 2 < NCH:
                    load_chunk_x(c + 2)
                elif c + 2 == NCH:
                    load_chunk_o(0)
                base = c * NC
                xr = [xr_tiles.pop((c, s)) for s in range(NSUB)]

                # transpose x (f32, 2c/r); evacuation casts to bf16
                xt = []
                for t in range(NT):
                    tp = psT.tile([P, NC], F32, name="tp", tag="psT")
                    for s in range(NSUB):
                        nc.tensor.transpose(
                            tp[:, s * P : (s + 1) * P],
                            xr[s][:, t * P : (t + 1) * P],
                            ident_f32,
                        )
                    xt_t = xtp.tile([P, NC], BF16, name="xt", tag="xt")
                    if t % 2 == 0:
                        nc.scalar.activation(xt_t[:], tp[:], ACTF.Copy)
                    else:
                        nc.vector.tensor_copy(xt_t[:], tp[:])
                    xt.append(xt_t)

                # mm1: encodedT[:, chunk] = tanh(W_enc^T xT + b_enc)
                for k in range(ND):
                    ps = psA.tile([P, NC], F32, name="ps1", tag="psA")
                    for t in range(NT):
                        nc.tensor.matmul(
                            ps[:], wenc[t][:, k * P : (k + 1) * P], xt[t][:],
                            start=(t == 0), stop=(t == NT - 1),
                        )
                    nc.scalar.activation(
                        encT[k][:, base : base + NC], ps[:], ACTF.Tanh, bias=bencT[k][:]
                    )

                # enc natural tiles + onehot + seg/cnt accumulation + nsq
                for s in range(NSUB):
                    i = c * NSUB + s
                    pse = psT.tile([P, D], BF16, name="pseb", tag="psT")
                    for k in range(ND):
                        nc.tensor.transpose(
                            pse[:, k * P : (k + 1) * P],
                            encT[k][:, base + s * P : base + (s + 1) * P],
                            ident_bf,
                        )
                    en = enp.tile([P, D], BF16, name="en", tag="en")
                    nc.vector.tensor_copy(en[:], pse[:])

                    nc.vector.tensor_scalar(
                        out=onehot_f[i][:], in0=iotaf[:], scalar1=labf_strip[:, i : i + 1],
                        scalar2=None, op0=ALU.is_equal,
                    )
                    nc.vector.tensor_copy(onehot_b[i][:], onehot_f[i][:])

                    nc.tensor.matmul(
                        seg_ps[:], onehot_b[i][:], en[:],
                        start=(i == 0), stop=(i == NN - 1),
                    )
                    nc.tensor.matmul(
                        cnt_ps[:], onehot_b[i][:], ones_col_bf[:],
                        start=(i == 0), stop=(i == NN - 1),
                    )

                    jn = junkp.tile([P, D], F32, name="jn", tag="junk")
                    nc.vector.scalar_tensor_tensor(
                        out=jn[:], in0=en[:], scalar=0.0, in1=en[:],
                        op0=ALU.bypass, op1=ALU.mult,
                        accum_out=nsq_strip[:, i : i + 1],
                    )

            # ============ collective #1: seg sums + counts ============
            seg_sb = accp.tile([C, D], F32)
            nc.vector.tensor_copy(seg_sb[:], seg_ps[:])
            cnt_sb = accp.tile([C, 1], F32)
            nc.vector.tensor_copy(cnt_sb[:], cnt_ps[:])

            bounce1_in = dp.tile([C, D + 1], F32, name="bounce1_in", tag="b1i")
            bounce1_out = dp.tile([C, D + 1], F32, name="bounce1_out", tag="b1o")
            nc.sync.dma_start(bounce1_in[:, 0:D], seg_sb[:])
            nc.sync.dma_start(bounce1_in[:, D : D + 1], cnt_sb[:])
            nc.gpsimd.collective_compute(
                "AllReduce",
                ALU.add,
                replica_groups=[list(range(NCORES))],
                ins=[bounce1_in[:].opt()],
                outs=[bounce1_out[:].opt()],
            )
            # ============ W_dd = W_dec @ W_enc (on device, bf16) ============
            wdecT = []
            for t in range(NT):
                psw = psT.tile([P, D], BF16, name="psw", tag="psT")
                for k in range(ND):
                    nc.tensor.transpose(
                        psw[:, k * P : (k + 1) * P],
                        wdec[k][:, t * P : (t + 1) * P],
                        ident_bf,
                    )
                wt_ = wdtp.tile([P, D], BF16, name="wdecT", tag=f"wdecT{t}")
                nc.vector.tensor_copy(wt_[:], psw[:])
                wdecT.append(wt_)
            W_dd = []
            for k in range(ND):
                psd = psA.tile([P, D], F32, name="psd", tag="psA")
                for t in range(NT):
                    nc.tensor.matmul(
                        psd[:], wdecT[t][:, k * P : (k + 1) * P], wenc[t][:],
                        start=(t == 0), stop=(t == NT - 1),
                    )
                wd_ = wts.tile([P, D], BF16, name="W_dd", tag=f"Wdd{k}")
                nc.scalar.activation(wd_[:], psd[:], ACTF.Copy)
                W_dd.append(wd_)

            # ================= phase B =================
            for c in range(NCH):
                base = c * NC

                if c + 1 < NCH:
                    load_chunk_o(c + 1)
                orow = [or_tiles.pop((c, s)) for s in range(NSUB)]

                # mm2 natural + rec pinball
                for s in range(NSUB):
                    i = c * NSUB + s
                    for q in range(NQ):
                        pool = psA if q % 2 == 0 else psT
                        ps = pool.tile([P, 512], F32, name="ps2", tag=pool.name)
                        if with_bdec:
                            nc.tensor.matmul(
                                ps[:], ones_k1bf[:], bdec16[:, q * 512 : (q + 1) * 512],
                                start=True, stop=False,
                            )
                        for k in range(ND):
                            nc.tensor.matmul(
                                ps[:],
                                encT[k][:, i * P : (i + 1) * P],
                                wdec[k][:, q * 512 : (q + 1) * 512],
                                start=(k == 0 and not with_bdec), stop=(k == ND - 1),
                            )
                        nc.vector.tensor_tensor(
                            ps[:], ps[:], orow[s][:, q * 512 : (q + 1) * 512],
                            ALU.subtract,
                        )
                        if (s + q) % 2 == 0:
                            nc.scalar.activation(
                                ps[:], ps[:], ACTF.Abs,
                                accum_out=rec_strip[:, i * NQ + q : i * NQ + q + 1],
                            )
                        else:
                            nc.vector.tensor_reduce(
                                rec_strip[:, i * NQ + q : i * NQ + q + 1], ps[:],
                                AX.X, ALU.add, apply_absolute_value=True,
                            )

                # mm4: rec_latentsT via W_dd; lat pinball vs encT
                for k2 in range(ND):
                    ps = psA.tile([P, NC], F32, name="ps4", tag="psA")
                    for k in range(ND):
                        nc.tensor.matmul(
                            ps[:],
                            W_dd[k][:, k2 * P : (k2 + 1) * P],
                            encT[k][:, base : base + NC],
                            start=(k == 0), stop=(k == ND - 1),
                        )
                    nc.scalar.activation(ps[:], ps[:], ACTF.Tanh, bias=bencT[k2][:])
                    nc.vector.tensor_tensor(
                        ps[:], ps[:], encT[k2][:, base : base + NC], ALU.subtract
                    )
                    nc.vector.tensor_reduce(
                        lat_strip[:, c * ND + k2 : c * ND + k2 + 1], ps[:],
                        AX.X, ALU.add, apply_absolute_value=True,
                    )

                # mm3 + CCE, blocked [128, 4*50]
                psl = psS.tile([P, NSUB * C], F32, name="psl", tag="psS")
                for a in range(NSUB):
                    sl = slice(a * C, (a + 1) * C)
                    nc.tensor.matmul(
                        psl[:, sl], ones_k1bf[:], bcls_row[:],
                        start=True, stop=False,
                    )
                    for k in range(ND):
                        nc.tensor.matmul(
                            psl[:, sl],
                            encT[k][:, (base + a * P) : (base + (a + 1) * P)],
                            wcls[k][:],
                            start=False, stop=(k == ND - 1),
                        )
                expt = ccep.tile([P, NSUB * C], F32, name="expt", tag="cce")
                nc.scalar.activation(expt[:], psl[:], ACTF.Exp)
                sume = smallp.tile([P, NSUB], F32, name="sume", tag="small")
                nc.vector.tensor_reduce(
                    sume[:], expt[:].rearrange("p (a c) -> p a c", c=C),
                    AX.X, ALU.add,
                )
                rcp = smallp.tile([P, NSUB], F32, name="rcp", tag="small")
                nc.vector.reciprocal(rcp[:], sume[:])

                cl = ccep.tile([P, NSUB * C], F32, name="cl", tag="cce")
                for a in range(NSUB):
                    nc.sync.dma_start(
                        cl[:, a * C : (a + 1) * C],
                        cl_d[(base + a * P) : (base + (a + 1) * P), :],
                    )
                rs = smallp.tile([P, NSUB], F32, name="rs", tag="small")
                nc.vector.tensor_reduce(
                    rs[:], cl[:].rearrange("p (a c) -> p a c", c=C),
                    AX.X, ALU.add,
                )
                lnrs = smallp.tile([P, NSUB], F32, name="lnrs", tag="small")
                nc.scalar.activation(lnrs[:], rs[:], ACTF.Ln)
                lg = ccep.tile([P, NSUB * C], F32, name="lg", tag="cce")
                nc.scalar.activation(lg[:], cl[:], ACTF.Ln)
                for a in range(NSUB):
                    i = c * NSUB + a
                    sl = slice(a * C, (a + 1) * C)
                    lgc = ccep.tile([P, C], F32, name="lgc", tag="lgc")
                    nc.vector.tensor_scalar(
                        out=lgc[:], in0=lg[:, sl], scalar1=lnrs[:, a : a + 1],
                        scalar2=LNEPS, op0=ALU.subtract, op1=ALU.max,
                    )
                    jc = ccep.tile([P, C], F32, name="jc", tag="lgc")
                    nc.vector.scalar_tensor_tensor(
                        out=jc[:], in0=expt[:, sl], scalar=rcp[:, a : a + 1],
                        in1=lgc[:], op0=ALU.mult, op1=ALU.mult,
                        accum_out=cat_strip[:, i : i + 1],
                    )

            # readback of collective #1 (emitted late so the in-order DMA
            # queue never blocks phase-B streaming loads on the collective)
            sums_g = accp.tile([C, D], F32)
            nc.sync.dma_start(sums_g[:], bounce1_out[:, 0:D])
            counts_g = accp.tile([C, 1], F32)
            nc.sync.dma_start(counts_g[:], bounce1_out[:, D : D + 1])

            # ============ collective #2: scalar partials ============
            pack3 = accp.tile([P, 3], F32)
            nc.vector.tensor_reduce(pack3[:, 0:1], rec_strip[:], AX.X, ALU.add)
            nc.vector.tensor_reduce(pack3[:, 1:2], lat_strip[:], AX.X, ALU.add)
            nc.vector.tensor_reduce(pack3[:, 2:3], cat_strip[:], AX.X, ALU.add)
            scps = psS.tile([1, 3], F32, name="scps", tag="psS")
            nc.tensor.matmul(scps[:], ones_col_f[:], pack3[:], start=True, stop=True)
            sc_row = accp.tile([1, 128], F32)
            nc.any.memset(sc_row[:], 0.0)
            nc.vector.tensor_copy(sc_row[:, 0:3], scps[:])

            bounce2_in = dp.tile([1, 128], F32, name="bounce2_in", tag="b2i")
            bounce2_out = dp.tile([1, 128], F32, name="bounce2_out", tag="b2o")
            nc.sync.dma_start(bounce2_in[:], sc_row[:])
            nc.gpsimd.collective_compute(
                "AllReduce",
                ALU.add,
                replica_groups=[list(range(NCORES))],
                ins=[bounce2_in[:].opt()],
                outs=[bounce2_out[:].opt()],
            )
            sc_g = accp.tile([1, 3], F32)
            nc.sync.dma_start(sc_g[:], bounce2_out[:, 0:3])

            # ================= phase C =================
            cmax = accp.tile([C, 1], F32)
            nc.vector.tensor_scalar(
                out=cmax[:], in0=counts_g[:], scalar1=1.0, scalar2=None, op0=ALU.max
            )
            crcp = accp.tile([C, 1], F32)
            nc.vector.reciprocal(crcp[:], cmax[:])
            means = accp.tile([C, D], F32)
            nc.vector.tensor_scalar(
                out=means[:], in0=sums_g[:], scalar1=crcp[:], scalar2=None, op0=ALU.mult
            )
            msq_col = accp.tile([C, 1], F32)
            jm = junkp.tile([C, D], F32, name="jm", tag="junk")
            nc.vector.scalar_tensor_tensor(
                out=jm[:], in0=means[:], scalar=0.0, in1=means[:],
                op0=ALU.bypass, op1=ALU.mult, accum_out=msq_col[:],
            )

            meansT = []
            for k in range(ND):
                tpm = psS.tile([P, C], F32, name="tpm", tag="psS")
                nc.tensor.transpose(
                    tpm[:], means[:, k * P : (k + 1) * P], ident_f32[:C, :C]
                )
                mt = accp.tile([P, C], BF16, name="meansT", tag=f"meansT{k}")
                nc.vector.tensor_copy(mt[:], tpm[:])
                meansT.append(mt)

            tpq = psS.tile([1, C], F32, name="tpq", tag="psS")
            nc.tensor.transpose(tpq[:], msq_col[:], ident_f32[:C, :C])
            msqm05_row = accp.tile([1, C], BF16)
            nc.scalar.activation(msqm05_row[:], tpq[:], ACTF.Copy, scale=-0.5)

            # eps' = enc @ means^T - 0.5*msq ; gq = -2 * sum_c oh * eps'
            for g in range(NN // NSUB):
                pse2 = psS.tile([P, NSUB * C], F32, name="pse2", tag="psS")
                for a in range(NSUB):
                    i = g * NSUB + a
                    sl = slice(a * C, (a + 1) * C)
                    nc.tensor.matmul(
                        pse2[:, sl], ones_k1bf[:], msqm05_row[:],
                        start=True, stop=False,
                    )
                    for k in range(ND):
                        nc.tensor.matmul(
                            pse2[:, sl],
                            encT[k][:, i * P : (i + 1) * P],
                            meansT[k][:],
                            start=False, stop=(k == ND - 1),
                        )
                for a in range(NSUB):
                    i = g * NSUB + a
                    sl = slice(a * C, (a + 1) * C)
                    jq = ccep.tile([P, C], F32, name="jq", tag="lgc")
                    nc.vector.scalar_tensor_tensor(
                        out=jq[:], in0=pse2[:, sl], scalar=-2.0, in1=onehot_f[i][:],
                        op0=ALU.mult, op1=ALU.mult,
                        accum_out=gq_strip[:, i : i + 1],
                    )

            # ---- final: S scalar + combine + output DMA ----
            coef = accp.tile([1, 3], F32)
            nc.any.memset(coef[:, 0:1], 0.9 / (n_global * T))
            nc.any.memset(coef[:, 1:2], 0.9 / (n_global * D))
            nc.any.memset(coef[:, 2:3], -1.0 / n_global)
            sprod = accp.tile([1, 3], F32)
            nc.vector.tensor_tensor(sprod[:], sc_g[:], coef[:], ALU.mult)
            stot = accp.tile([1, 1], F32)
            nc.vector.tensor_reduce(stot[:], sprod[:], AX.X, ALU.add)
            psS1 = psS.tile([P, 1], F32, name="psS1", tag="psS")
            nc.tensor.matmul(psS1[:], ones_k1f[:], stot[:], start=True, stop=True)
            s_col = accp.tile([P, 1], F32)
            nc.vector.tensor_copy(s_col[:], psS1[:])

            ns2 = accp.tile([P, NN], F32)
            nc.vector.tensor_scalar(
                out=ns2[:], in0=nsq_strip[:], scalar1=1.0 / D, scalar2=s_col[:],
                op0=ALU.mult, op1=ALU.add,
            )
            out_strip = accp.tile([P, NN], F32)
            nc.vector.scalar_tensor_tensor(
                out=out_strip[:], in0=gq_strip[:], scalar=1.0 / D, in1=ns2[:],
                op0=ALU.mult, op1=ALU.add,
            )
            ps_out = psS.tile([NN, P], F32, name="ps_out", tag="psS")
            nc.tensor.transpose(ps_out[:], out_strip[:], ident_f32)
            outT = accp.tile([NN, P], F32)
            nc.vector.tensor_copy(outT[:], ps_out[:])
            nc.sync.dma_start(
                out_d.ap().rearrange("(a p) -> a p", p=P), outT[:]
            )

    nc.compile()
    return nc


_CACHE = {}


def _get_nc(with_bdec=True):
    key = ("nc", with_bdec)
    if key not in _CACHE:
        _CACHE[key] = build(with_bdec=with_bdec)
    return _CACHE[key]


def kernel(**inputs):
    nc = _get_nc(with_bdec=bool(np.any(inputs["b_dec"])))
    nl = N_GLOBAL // NCORES
    shard_names = ["x", "output", "cat_labels", "labels"]
    full_names = ["W_enc", "b_enc", "W_dec", "b_dec", "W_cls", "b_cls"]
    in_maps = []
    for i in range(NCORES):
        m = {}
        for k in shard_names:
            m[k] = np.ascontiguousarray(inputs[k][i * nl : (i + 1) * nl])
        for k in full_names:
            m[k] = np.ascontiguousarray(inputs[k])
        in_maps.append(m)
    res = run_bass_kernel_spmd(nc, in_maps, list(range(NCORES))).results
    return np.concatenate([res[i]["out"] for i in range(NCORES)]).astype(np.float32)
